# revision 10
# baseline (speedup 1.0000x reference)
"""Trainium2 Bass kernel for KipfAndWillingConv (GNN message passing), v4.

v3 + rank-paired tile slots (each core's tiles sorted by edge count so
the shared-NEFF cross-core max padding nearly vanishes), bf16 output
(host casts back to fp32), and 2-tile gathered-stream DMA batches
alternating across both HWDGE rings.
"""

import numpy as np
import ml_dtypes

N_NODES = 100000
N_FEAT = 512
N_FILT = 512
N_CORES = 8
ROWS_PER_CORE = N_NODES // N_CORES      # 12500
TILE = 128
N_TILES = (ROWS_PER_CORE + TILE - 1) // TILE   # 98
NBAND = 4
BW = 32

BF16 = ml_dtypes.bfloat16
FP8 = ml_dtypes.float8_e3m4

TRACE = False
LAST_RESULTS = None


def _prepare(x, filters, edge_src, edge_dst, edge_weight):
    E = edge_src.shape[0]
    core = edge_dst // ROWS_PER_CORE
    dst_local = edge_dst - core * ROWS_PER_CORE
    tile_id = dst_local >> 7
    row = (dst_local & 127).astype(np.int64)

    ctr = (core.astype(np.int64) * N_TILES + tile_id) * TILE + row
    cnt = np.bincount(ctr, minlength=N_CORES * N_TILES * TILE).reshape(
        N_CORES, N_TILES, TILE)

    # greedy balance rows into 4 bands of 32 rows per (core, tile); bands
    # relabeled by descending sum so band ranks align across cores
    band_of_row = np.empty((N_CORES, N_TILES, TILE), np.int64)
    rank_in_band = np.empty((N_CORES, N_TILES, TILE), np.int64)
    band_cnt = np.empty((N_CORES, N_TILES, NBAND), np.int64)
    for c in range(N_CORES):
        for t in range(N_TILES):
            order = np.argsort(-cnt[c, t], kind="stable")
            sums = [0] * NBAND
            nrows = [0] * NBAND
            bor = np.empty(TILE, np.int64)
            rib = np.empty(TILE, np.int64)
            for r in order:
                b = min((bb for bb in range(NBAND) if nrows[bb] < BW),
                        key=lambda bb: sums[bb])
                bor[r] = b
                rib[r] = nrows[b]
                sums[b] += cnt[c, t, r]
                nrows[b] += 1
            relabel = np.argsort(np.argsort(-np.asarray(sums), kind="stable"))
            band_of_row[c, t] = relabel[bor]
            rank_in_band[c, t] = rib
            band_cnt[c, t] = np.asarray(sums)[np.argsort(-np.asarray(sums))]

    # rank-paired slots: each core sorts its tiles by edge count desc
    tile_tot = band_cnt.sum(axis=2)                     # [C, T]
    order_c = np.argsort(-tile_tot, axis=1, kind="stable")   # slot -> tile
    slot_of_tile = np.argsort(order_c, axis=1)               # tile -> slot
    # band counts arranged by slot: bc_slot[c, s, b]
    bc_slot = np.take_along_axis(band_cnt, order_c[:, :, None], axis=1)
    # tail-merge: per band only floor(max/128) full chunks; overflow of all
    # 4 bands shares 128-wide tail chunks (one-hot over the full 128 rows)
    F = np.maximum(1, bc_slot.max(axis=0) // TILE)      # [S, NBAND]
    ov = np.maximum(0, bc_slot - F[None] * TILE)        # [C, S, NBAND]
    ov_tot = ov.sum(axis=2)                             # [C, S]
    Tn = np.maximum(1, np.ceil(ov_tot.max(axis=0) / TILE)).astype(np.int64)
    base = np.zeros(N_TILES + 1, np.int64)
    np.cumsum(F.sum(axis=1) + Tn, out=base[1:])
    off_full = base[:-1, None] + np.cumsum(
        np.concatenate([np.zeros((N_TILES, 1), np.int64), F[:, :3]], axis=1),
        axis=1)                                         # [S, NBAND]
    off_tail = base[:-1] + F.sum(axis=1)                # [S]
    NCH = int(base[-1])
    nts = (F.tolist(), Tn.tolist())
    tailbase = np.concatenate(
        [np.zeros((N_CORES, N_TILES, 1), np.int64),
         np.cumsum(ov[:, :, :3], axis=2)], axis=2)      # [C, S, NBAND]

    # per-edge slot in the stream
    eslot = slot_of_tile[core, tile_id]
    eb = band_of_row[core, tile_id, row]
    erank = rank_in_band[core, tile_id, row]
    key = (core.astype(np.int64) * N_TILES + eslot) * NBAND + eb
    order = np.argsort(key, kind="stable")
    key_s = key[order]
    kcounts = np.bincount(key_s, minlength=N_CORES * N_TILES * NBAND)
    starts = np.zeros(N_CORES * N_TILES * NBAND + 1, np.int64)
    np.cumsum(kcounts, out=starts[1:])
    pos = np.arange(E, dtype=np.int64) - starts[key_s]
    sb_s = key_s % (N_TILES * NBAND)
    core_s = key_s // (N_TILES * NBAND)
    es_ = sb_s // NBAND
    eb_ = sb_s % NBAND
    cap = F[es_, eb_] * TILE
    is_tail = pos >= cap
    slot = np.where(
        is_tail,
        off_tail[es_] * TILE + tailbase[core_s, es_, eb_] + (pos - cap),
        off_full[es_, eb_] * TILE + pos)

    src_s = edge_src[order].astype(np.int64)
    rank_s = erank[order]
    rank_s = np.where(is_tail, eb_ * BW + rank_s, rank_s)
    w_s = edge_weight[order]

    tail_cols = np.concatenate([
        np.arange(base[s] + int(np.sum(F[s])), base[s + 1])
        for s in range(N_TILES)]).astype(np.int64)

    x_q = np.ascontiguousarray(x.astype(FP8))
    w_img = np.ascontiguousarray(
        filters.reshape(4, 128, N_FILT).transpose(1, 0, 2).reshape(128, 4 * N_FILT)
    ).astype(BF16)
    eye = np.eye(128, dtype=BF16)
    iota = np.ascontiguousarray(
        np.broadcast_to(np.arange(BW, dtype=np.float32), (128, BW))).astype(BF16)
    iota128 = np.ascontiguousarray(
        np.broadcast_to(np.arange(128, dtype=np.float32), (128, 128))).astype(BF16)

    in_maps = []
    for c in range(N_CORES):
        msk = core_s == c
        slot_c = slot[msk]
        idx_pad = np.zeros(NCH * TILE, np.int64)
        idx_pad[slot_c] = src_s[msk]
        row_pad = np.zeros(NCH * TILE, np.float32)
        row_pad[slot_c] = rank_s[msk]
        w_pad = np.zeros(NCH * TILE, np.float32)
        w_pad[slot_c] = w_s[msk]

        xg = x_q[idx_pad].reshape(NCH, TILE, N_FEAT)
        xg_dev = np.ascontiguousarray(xg.transpose(1, 0, 2)).reshape(
            TILE, NCH * N_FEAT)
        meta = np.empty((TILE, 2 * NCH), np.float32)
        meta[:, :NCH] = row_pad.reshape(NCH, TILE).T
        meta[:, NCH:] = w_pad.reshape(NCH, TILE).T
        metab = np.ascontiguousarray(meta.astype(BF16))
        metat = np.ascontiguousarray(
            np.concatenate([meta[:, tail_cols], meta[:, NCH + tail_cols]],
                           axis=1))

        in_maps.append({
            "xg": xg_dev, "metab": metab, "metat": metat, "wmat": w_img,
            "eye": eye, "iota": iota, "iota128": iota128,
        })

    # map original row (t*128+r) -> device row (slot*128 + 32*band + rank)
    dev_idx = np.empty((N_CORES, N_TILES * TILE), np.int64)
    for c in range(N_CORES):
        t_idx = np.repeat(np.arange(N_TILES), TILE)
        r_idx = np.tile(np.arange(TILE), N_TILES)
        dev_idx[c] = (slot_of_tile[c, t_idx] * TILE
                      + band_of_row[c, t_idx, r_idx] * BW
                      + rank_in_band[c, t_idx, r_idx])
    return in_maps, nts, dev_idx


def _build(nts):
    import concourse.bacc as bacc
    import concourse.mybir as mybir
    import concourse.tile as tile
    from concourse._compat import get_trn_type

    F_l, T_l = nts
    NCH = int(np.sum(F_l) + np.sum(T_l))
    f32 = mybir.dt.float32
    bf16 = mybir.dt.bfloat16
    fp8 = mybir.dt.float8e3
    EQ = mybir.AluOpType.is_equal
    MULT = mybir.AluOpType.mult

    nc = bacc.Bacc(get_trn_type() or "TRN2", target_bir_lowering=False, debug=False)
    xg_d = nc.dram_tensor("xg", [TILE, NCH * N_FEAT], fp8, kind="ExternalInput")
    NT = int(np.sum(T_l))
    metab_d = nc.dram_tensor("metab", [TILE, 2 * NCH], bf16, kind="ExternalInput")
    metat_d = nc.dram_tensor("metat", [TILE, 2 * NT], f32, kind="ExternalInput")
    w_d = nc.dram_tensor("wmat", [128, 4 * N_FILT], bf16, kind="ExternalInput")
    eye_d = nc.dram_tensor("eye", [128, 128], bf16, kind="ExternalInput")
    iota_d = nc.dram_tensor("iota", [128, BW], bf16, kind="ExternalInput")
    iota128_d = nc.dram_tensor("iota128", [128, 128], bf16, kind="ExternalInput")
    out_d = nc.dram_tensor("out", [N_TILES * 128, N_FILT], bf16,
                           kind="ExternalOutput")

    with tile.TileContext(nc) as tc:
        with (
            tc.tile_pool(name="const", bufs=1) as pc,
            tc.tile_pool(name="gath", bufs=4) as pg,
            tc.tile_pool(name="ohp", bufs=4) as poh,
            tc.tile_pool(name="eqp", bufs=2) as peq,
            tc.tile_pool(name="sp", bufs=2) as ps_pool,
            tc.tile_pool(name="stp", bufs=2) as pst_pool,
            tc.tile_pool(name="outp", bufs=2) as pout,
            tc.tile_pool(name="psS", bufs=3, space="PSUM") as ppsS,
            tc.tile_pool(name="psT", bufs=2, space="PSUM") as ppsT,
            tc.tile_pool(name="psO", bufs=3, space="PSUM") as ppsO,
        ):
            w_sb = pc.tile([128, 4 * N_FILT], bf16)
            nc.sync.dma_start(w_sb[:], w_d[:])
            eye_sb = pc.tile([128, 128], bf16)
            nc.sync.dma_start(eye_sb[:], eye_d[:])
            iota_sb = pc.tile([128, BW], bf16)
            nc.sync.dma_start(iota_sb[:], iota_d[:])
            iota128_sb = pc.tile([128, 128], bf16)
            nc.sync.dma_start(iota128_sb[:], iota128_d[:])
            metab_sb = pc.tile([TILE, 2 * NCH], bf16)
            nc.sync.dma_start(metab_sb[:], metab_d[:])
            metat_sb = pc.tile([TILE, 2 * NT], f32)
            nc.sync.dma_start(metat_sb[:], metat_d[:])

            # 2-tile DMA batches for the gathered stream
            pair_start = list(range(0, N_TILES, 2))
            ch_of_slot = np.zeros(N_TILES + 1, np.int64)
            np.cumsum([int(sum(F_l[s]) + T_l[s]) for s in range(N_TILES)],
                      out=ch_of_slot[1:])
            g_tiles = {}
            for p, s0 in enumerate(pair_start):
                s1 = min(s0 + 2, N_TILES)
                c0, c1 = int(ch_of_slot[s0]), int(ch_of_slot[s1])
                eng = nc.sync if (p % 2 == 0) else nc.scalar
                g2 = pg.tile([128, (c1 - c0) * N_FEAT], fp8, tag="g")
                eng.dma_start(g2[:], xg_d[:, c0 * N_FEAT:c1 * N_FEAT])
                for s in range(s0, s1):
                    g_tiles[s] = (g2, int(ch_of_slot[s]) - c0)

            tail0 = 0
            for t in range(N_TILES):
                nbs = [int(v) for v in F_l[t]]       # full chunks per band
                ntail = int(T_l[t])
                nfull = int(sum(nbs))
                ncht = nfull + ntail
                ch0 = int(ch_of_slot[t])
                boff = [int(sum(nbs[:b])) for b in range(NBAND)]
                g_t, gch0 = g_tiles[t]
                # oh layout: nfull chunks of width BW, then ntail of width 128
                oh_t = poh.tile([128, nfull * BW + ntail * 128], bf16, tag="oh")
                # all full-band one-hots in two whole-tile TT ops via
                # broadcast (step-0) access patterns
                eq_t = peq.tile([128, nfull * BW], bf16, tag="eq")
                iota3 = iota_sb[:].unsqueeze(1).broadcast_to((128, nfull, BW))
                rows3 = metab_sb[:, ch0:ch0 + nfull].unsqueeze(2).broadcast_to(
                    (128, nfull, BW))
                w3 = metab_sb[:, NCH + ch0:NCH + ch0 + nfull].unsqueeze(2).broadcast_to(
                    (128, nfull, BW))
                nc.vector.tensor_tensor(
                    eq_t[:].rearrange("p (n r) -> p n r", r=BW), iota3, rows3, EQ)
                nc.vector.tensor_tensor(
                    oh_t[:, :nfull * BW].rearrange("p (n r) -> p n r", r=BW),
                    eq_t[:].rearrange("p (n r) -> p n r", r=BW), w3, MULT)
                for j in range(ntail):
                    g = tail0 + j
                    o0 = nfull * BW + j * 128
                    nc.vector.tensor_scalar(
                        oh_t[:, o0:o0 + 128],
                        iota128_sb[:],
                        metat_sb[:, g:g + 1],
                        metat_sb[:, NT + g:NT + g + 1],
                        EQ, MULT,
                    )
                tail0 += ntail
                psS = ppsS.tile([128, 512], f32)
                for i in range(max(nbs)):
                    for b in range(NBAND):
                        if i >= nbs[b]:
                            continue
                        ch = boff[b] + i
                        nc.tensor.matmul(
                            psS[BW * b:BW * (b + 1), :],
                            oh_t[:, ch * BW:(ch + 1) * BW],
                            g_t[:, (gch0 + ch) * N_FEAT:(gch0 + ch + 1) * N_FEAT],
                            start=(i == 0), stop=False,
                            tile_position=(0, BW * b),
                            skip_group_check=True,
                        )
                # tail chunks: the 128-wide one-hot's column block b holds
                # exactly band b's rows, so emit as 4 col-tiled matmuls
                # sharing one rhs chunk (uniform tile geometry)
                for j in range(ntail):
                    ch = nfull + j
                    o0 = nfull * BW + j * 128
                    for b in range(NBAND):
                        nc.tensor.matmul(
                            psS[BW * b:BW * (b + 1), :],
                            oh_t[:, o0 + BW * b:o0 + BW * (b + 1)],
                            g_t[:, (gch0 + ch) * N_FEAT:(gch0 + ch + 1) * N_FEAT],
                            start=False, stop=(j == ntail - 1),
                            tile_position=(0, BW * b),
                            skip_group_check=True,
                        )
                s_t = ps_pool.tile([128, 512], bf16)
                nc.scalar.copy(s_t[:], psS[:])
                psT = ppsT.tile([128, 512], bf16)
                for k in range(4):
                    nc.tensor.transpose(
                        psT[:, k * 128:(k + 1) * 128],
                        s_t[:, k * 128:(k + 1) * 128],
                        eye_sb[:],
                    )
                st_t = pst_pool.tile([128, 512], bf16)
                nc.scalar.copy(st_t[:], psT[:])
                psO = ppsO.tile([128, 512], f32)
                for k in range(4):
                    nc.tensor.matmul(
                        psO[:],
                        st_t[:, k * 128:(k + 1) * 128],
                        w_sb[:, k * N_FILT:(k + 1) * N_FILT],
                        start=(k == 0), stop=(k == 3),
                    )
                o_t = pout.tile([128, 512], bf16)
                nc.scalar.copy(o_t[:], psO[:])
                eng2 = nc.scalar if (t % 2 == 0) else nc.sync
                eng2.dma_start(out_d[t * 128:(t + 1) * 128, :], o_t[:])

    nc.compile()
    return nc


def kernel(x, filters, edge_src, edge_dst, edge_weight):
    global LAST_RESULTS
    from concourse import bass_utils

    in_maps, nts, dev_idx = _prepare(x, filters, edge_src, edge_dst,
                                     edge_weight)
    nc = _build(nts)
    res = bass_utils.run_bass_kernel_spmd(
        nc, in_maps, list(range(N_CORES)), trace=TRACE,
    )
    LAST_RESULTS = res
    outs = []
    for c in range(N_CORES):
        outs.append(res.results[c]["out"][dev_idx[c, :ROWS_PER_CORE]])
    return np.ascontiguousarray(
        np.concatenate(outs, axis=0)).astype(np.float32)
